# revision 22
# baseline (speedup 1.0000x reference)
"""Slot-attention module Bass/Tile kernel (nn_AttentionModule_39084202394083).

Contract: kernel(**inputs) takes FULL unsharded inputs (B=64, N=4096, D=256,
S=8 slots, 3 iterations) and returns the FULL output [S=8, B=64, D=256] f32.

Sharding: data-parallel over batch B across 8 NeuronCores (8 batch elements
per core, all params replicated); no collectives.

Design (v2 — reassociated attention, no k/v materialization):
  - All LN affine params and projection chains are folded on the HOST:
      dots   = x^T.T @ (wkp @ q^T)        (k never materialized)
      gru_x  = ((E@x_hat)/den) @ (wvp @ w_ih) + bias   (v never materialized)
    so the only big on-device tensors are x_hat (LN'd input, natural layout,
    with a ones column -> attention denominators fall out of the E@x matmul)
    and x^T (for the dots stationary operand), both bf16.
  - dots are computed transposed [N, 8] so softmax over slots is a free-dim
    reduce; exp folds the 1/sqrt(D) scale; sigma is applied to E^T via a
    free-dim-broadcast tensor_tensor multiply.
  - Slot state lives at 32-spaced partition rows (row = 32*b_in_round + s),
    so the E@x matmuls for 4 batches col-tile into PSUM partition groups via
    tile_position, and all slot-side GEMMs/element-wise ops run batched on
    [128, *] tiles with junk lanes zeroed.
  - SBUF->SBUF DMA xbar transposes build x^T; they alternate between the two
    HWDGE engines (sync + scalar) to halve the serial dispatch cost.
  - Batches run in 2 rounds of 4 so bf16 x_hat/x^T for a round fits SBUF.
"""

import numpy as np
import ml_dtypes
from contextlib import ExitStack

import concourse.bass as bass
import concourse.tile as tile
from concourse import bacc, mybir
from concourse.bass import ts
from concourse.bass_utils import run_bass_kernel_spmd
from concourse.masks import make_identity

F32 = mybir.dt.float32
BF16 = mybir.dt.bfloat16
AF = mybir.ActivationFunctionType
ALU = mybir.AluOpType

B_LOC = 8          # batch elements per core
N = 4096           # positions
D = 256            # model dim
S = 8              # slots
H = 1024           # mlp hidden
NT = N // 128      # 32 position tiles
KD = 2             # 128-chunks of D
KH = H // 128      # 8 128-chunks of H
ITERS = 3
ROUND = 4          # batches per round
NROUND = B_LOC // ROUND
SUP = 4            # position tiles per load super-tile
LN_EPS = 1e-5
SCALE = float(D) ** -0.5

BF = ml_dtypes.bfloat16

# folded-weight dram tensors: name -> (shape, dtype)
FW_NAMES = {
    "wkpT": ([D, D], BF16),      # (diag(g_in) wk)^T
    "wqp": ([D, D], BF16),       # diag(g_sl) wq
    "wvih": ([D, 3 * D], BF16),  # (diag(g_in) wv) @ w_ih
    "whh": ([D, 3 * D], BF16),
    "w1p": ([D, H], BF16),       # diag(g_ff) mlp_w1
    "w2": ([H, D], BF16),
    "gxbias": ([3 * D], BF16),   # (b_in@wv + bv)@w_ih + b_ih
    "ghbias": ([3 * D], BF16),   # b_hh
}
# optional (only shipped when nonzero): bkp [D], bqp [D], b1p [H], b2 [D]


def _build(flags):
    has_kbias, has_qbias, has_b1, has_b2 = flags
    nc = bacc.Bacc("TRN2", debug=False, enable_asserts=False)
    inp = nc.dram_tensor("inputs", [B_LOC, N, D], F32, kind="ExternalInput").ap()
    slots_in = nc.dram_tensor("slots", [S, B_LOC, D], F32, kind="ExternalInput").ap()
    W = {}
    for name, (shape, dt_) in FW_NAMES.items():
        W[name] = nc.dram_tensor(name, shape, dt_, kind="ExternalInput").ap()
    if has_kbias:
        W["bkp"] = nc.dram_tensor("bkp", [D], F32, kind="ExternalInput").ap()
    if has_qbias:
        W["bqp"] = nc.dram_tensor("bqp", [D], F32, kind="ExternalInput").ap()
    if has_b1:
        W["b1p"] = nc.dram_tensor("b1p", [H], F32, kind="ExternalInput").ap()
    if has_b2:
        W["b2"] = nc.dram_tensor("b2", [D], BF16, kind="ExternalInput").ap()
    out_dram = nc.dram_tensor("out", [S, B_LOC, D], F32, kind="ExternalOutput").ap()

    with tile.TileContext(nc) as tc:
        with ExitStack() as ctx:
            _body(ctx, tc, inp, slots_in, W, out_dram, flags)
    nc.compile()
    return nc


def _body(ctx, tc, inp, slots_in, W, out_dram, flags):
    has_kbias, has_qbias, has_b1, has_b2 = flags
    nc = tc.nc

    wts = ctx.enter_context(tc.tile_pool(name="wts", bufs=1))
    ps = ctx.enter_context(tc.tile_pool(name="ps", bufs=2, space="PSUM"))
    psg = ctx.enter_context(tc.tile_pool(name="psg", bufs=1, space="PSUM"))

    # ---------------- constants / weights (host-folded) ----------------
    id_f = wts.tile([128, 128], F32, tag="idf")
    make_identity(nc, id_f)
    id_b = wts.tile([128, 128], BF16, tag="idb")
    make_identity(nc, id_b)
    ones_row = wts.tile([1, 128], BF16, tag="ones_row")
    nc.vector.memset(ones_row, 1.0)
    eps_col = wts.tile([128, 1], F32, tag="eps_col")
    nc.vector.memset(eps_col, LN_EPS)

    def wload(name, kchunks, cols, dt_=BF16):
        t = wts.tile([128, kchunks, cols], dt_, tag="w_" + name)
        nc.sync.dma_start(out=t, in_=W[name].rearrange("(k p) c -> p k c", p=128))
        return t

    def wrow(name, n):
        t = wts.tile([1, n], BF16, tag="w_" + name)
        nc.sync.dma_start(out=t, in_=W[name].rearrange("(one a) -> one a", one=1))
        return t

    wkpT_b = wload("wkpT", KD, D)
    wqp_b = wload("wqp", KD, D)
    wvih_b = wload("wvih", KD, 3 * D)
    whh_b = wload("whh", KD, 3 * D)
    w1p_b = wload("w1p", KD, H)
    w2_b = wload("w2", KH, D)
    gxbias_row = wrow("gxbias", 3 * D)
    ghbias_row = wrow("ghbias", 3 * D)
    bkp_col = wload("bkp", KD, 1, F32) if has_kbias else None
    bqp_col = wload("bqp", KD, 1, F32) if has_qbias else None
    b1p_col = wload("b1p", KH, 1, F32) if has_b1 else None
    b2_row = wrow("b2", D) if has_b2 else None

    kvx = ctx.enter_context(tc.tile_pool(name="kvx", bufs=ROUND))
    slot_pool = ctx.enter_context(tc.tile_pool(name="slot", bufs=NROUND))
    xload = ctx.enter_context(tc.tile_pool(name="xload", bufs=2))
    stat = ctx.enter_context(tc.tile_pool(name="stat", bufs=2))
    itp = ctx.enter_context(tc.tile_pool(name="itp", bufs=2))
    sp = ctx.enter_context(tc.tile_pool(name="sp", bufs=1))

    def phase_a(b):
        """Load + LN + cast: returns (x_aug [128,NT,257] bf16, xT [128,KD,N] bf16)."""
        x_aug = kvx.tile([128, NT, D + 1], BF16, tag="xa")
        nc.vector.memset(x_aug[:, :, D:D + 1], 1.0)
        xT = kvx.tile([128, KD, N], BF16, tag="xT")
        inp_b = inp[b].rearrange("(c p) d -> p c d", p=128)
        mv_all = stat.tile([128, NT, 2], F32, tag="mv")
        r_all = stat.tile([128, NT, 1], F32, tag="r")
        negmr = stat.tile([128, NT, 1], F32, tag="negmr")

        for g in range(NT // SUP):
            xs = xload.tile([128, SUP, D], F32, tag="xs")
            nc.sync.dma_start(out=xs, in_=inp_b[:, ts(g, SUP), :])
            st6 = stat.tile([128, SUP, 6], F32, tag="st6")
            for j in range(SUP):
                nc.vector.bn_stats(out=st6[:, j, :], in_=xs[:, j, :])
            for j in range(SUP):
                nc.vector.bn_aggr(out=mv_all[:, g * SUP + j, :], in_=st6[:, j, :])
            nc.scalar.activation(out=r_all[:, ts(g, SUP), :],
                                 in_=mv_all[:, ts(g, SUP), 1:2],
                                 func=AF.Sqrt, bias=eps_col, scale=1.0)
            nc.vector.reciprocal(out=r_all[:, ts(g, SUP), :],
                                 in_=r_all[:, ts(g, SUP), :])
            nc.vector.tensor_tensor(out=negmr[:, ts(g, SUP), :],
                                    in0=mv_all[:, ts(g, SUP), 0:1],
                                    in1=r_all[:, ts(g, SUP), :], op=ALU.mult)
            nc.vector.tensor_scalar(out=negmr[:, ts(g, SUP), :],
                                    in0=negmr[:, ts(g, SUP), :],
                                    scalar1=-1.0, scalar2=None, op0=ALU.mult)
            for j in range(SUP):
                t = g * SUP + j
                if j % 2 == 0:
                    nc.vector.tensor_scalar(out=x_aug[:, t, 0:D],
                                            in0=xs[:, j, :],
                                            scalar1=mv_all[:, t, 0:1],
                                            scalar2=r_all[:, t, 0:1],
                                            op0=ALU.subtract, op1=ALU.mult)
                else:
                    nc.scalar.activation(out=x_aug[:, t, 0:D], in_=xs[:, j, :],
                                         func=AF.Identity,
                                         bias=negmr[:, t, :],
                                         scale=r_all[:, t, 0:1])
                for kd in range(KD):
                    pst = ps.tile([128, 128], BF16, tag="small")
                    nc.tensor.transpose(out=pst, in_=x_aug[:, t, ts(kd, 128)],
                                        identity=id_b)
                    if kd == 0:
                        nc.vector.tensor_copy(out=xT[:, kd, ts(t, 128)], in_=pst)
                    else:
                        nc.scalar.copy(out=xT[:, kd, ts(t, 128)], in_=pst)
        return x_aug, xT

    def pe_t(dst, src, identity):
        """dst[128, KD, 128] (bf16) = src[128, 256].T via PE transpose."""
        dt_ = src.dtype
        for j in range(KD):
            pst = ps.tile([128, 128], dt_, tag="small")
            nc.tensor.transpose(out=pst, in_=src[:, ts(j, 128)], identity=identity)
            nc.vector.tensor_copy(out=dst[:, j, :], in_=pst)

    def slot_ln_cast(sl, out_bf):
        st6 = sp.tile([128, 6], F32, tag="sst6")
        mv = sp.tile([128, 2], F32, tag="smv")
        rr = sp.tile([128, 1], F32, tag="srr")
        nc.vector.bn_stats(out=st6, in_=sl)
        nc.vector.bn_aggr(out=mv, in_=st6)
        nc.scalar.activation(out=rr, in_=mv[:, 1:2], func=AF.Sqrt,
                             bias=eps_col, scale=1.0)
        nc.vector.reciprocal(out=rr, in_=rr)
        nc.vector.tensor_scalar(out=out_bf, in0=sl, scalar1=mv[:, 0:1],
                                scalar2=rr, op0=ALU.subtract, op1=ALU.mult)

    for rnd in range(NROUND):
        slots_r = slot_pool.tile([128, D], F32, tag="slots")
        nc.vector.memset(slots_r, 0.0)
        for bl in range(ROUND):
            nc.sync.dma_start(out=slots_r[32 * bl:32 * bl + S, :],
                              in_=slots_in[:, rnd * ROUND + bl, :])
        xas, xTs = [], []
        for bl in range(ROUND):
            xa, xT = phase_a(rnd * ROUND + bl)
            xas.append(xa)
            xTs.append(xT)

        for it in range(ITERS):
            # ---- q^T then wqk = wkp @ q^T  (both [128, KD, 128] bf16) ----
            sn = sp.tile([128, D], BF16, tag="sn")
            slot_ln_cast(slots_r, sn)
            snT = sp.tile([128, KD, 128], BF16, tag="snT")
            pe_t(snT, sn, id_b)
            qT = sp.tile([128, KD, 128], BF16, tag="qT")
            for dt in range(KD):
                psq = ps.tile([128, 128], F32, tag="small")
                for kd in range(KD):
                    nc.tensor.matmul(psq, lhsT=wqp_b[:, kd, ts(dt, 128)],
                                     rhs=snT[:, kd, :],
                                     start=(kd == 0), stop=(kd == KD - 1))
                if has_qbias:
                    nc.scalar.activation(out=qT[:, dt, :], in_=psq,
                                         func=AF.Identity,
                                         bias=bqp_col[:, dt, :], scale=1.0)
                else:
                    nc.vector.tensor_copy(out=qT[:, dt, :], in_=psq)
            wqk = sp.tile([128, KD, 128], BF16, tag="wqk")
            for dt in range(KD):
                pwq = ps.tile([128, 128], F32, tag="small")
                for kd in range(KD):
                    nc.tensor.matmul(pwq, lhsT=wkpT_b[:, kd, ts(dt, 128)],
                                     rhs=qT[:, kd, :],
                                     start=(kd == 0), stop=(kd == KD - 1))
                nc.vector.tensor_copy(out=wqk[:, dt, :], in_=pwq)
            if has_kbias:
                pqb = ps.tile([1, 128], F32, tag="small")
                for kd in range(KD):
                    nc.tensor.matmul(pqb, lhsT=bkp_col[:, kd, :],
                                     rhs=qT[:, kd, :],
                                     start=(kd == 0), stop=(kd == KD - 1))
                qb_row = sp.tile([1, 128], BF16, tag="qb_row")
                nc.vector.tensor_copy(out=qb_row, in_=pqb)

            # ---- per batch: dots^T -> softmax -> E^T ----
            ets = []
            for bl in range(ROUND):
                dps = ps.tile([128, NT, S], F32, tag="dps")
                for t in range(NT):
                    for kd in range(KD):
                        nc.tensor.matmul(dps[:, t, :],
                                         lhsT=xTs[bl][:, kd, ts(t, 128)],
                                         rhs=wqk[:, kd, 32 * bl:32 * bl + S],
                                         start=(kd == 0),
                                         stop=(kd == KD - 1 and not has_kbias))
                    if has_kbias:
                        nc.tensor.matmul(dps[:, t, :], lhsT=ones_row,
                                         rhs=qb_row[:, 32 * bl:32 * bl + S],
                                         start=False, stop=True)
                et = itp.tile([128, NT, S], BF16, tag="et%d" % bl)
                nc.scalar.activation(out=et, in_=dps, func=AF.Exp, bias=0.0,
                                     scale=SCALE)
                sig = itp.tile([128, NT, 1], F32, tag="sig")
                nc.vector.tensor_reduce(out=sig, in_=et,
                                        axis=mybir.AxisListType.X, op=ALU.add)
                nc.vector.reciprocal(out=sig, in_=sig)
                nc.vector.tensor_tensor(out=et, in0=et,
                                        in1=sig.to_broadcast([128, NT, S]),
                                        op=ALU.mult)
                ets.append(et)

            # ---- eu = E^T.T @ x_aug, col-tiled 2 batches per PSUM bank ----
            eu_sb = sp.tile([128, D], BF16, tag="eu_sb")
            nc.vector.memset(eu_sb, 0.0)
            den_t = sp.tile([128, 1], F32, tag="den_t")
            for pair in range(ROUND // 2):
                eup = ps.tile([128, D + 1], F32, tag="eu")
                for t in range(NT):
                    for g in range(2):
                        bl = 2 * pair + g
                        r0 = 32 * bl
                        nc.tensor.matmul(eup[r0:r0 + S, :],
                                         lhsT=ets[bl][:, t, :],
                                         rhs=xas[bl][:, t, :],
                                         start=(t == 0), stop=(t == NT - 1),
                                         tile_position=(0, r0),
                                         skip_group_check=True)
                for g in range(2):
                    r0 = 32 * (2 * pair + g)
                    nc.vector.reciprocal(out=den_t[r0:r0 + S, :],
                                         in_=eup[r0:r0 + S, D:D + 1])
                    nc.vector.tensor_scalar(out=eu_sb[r0:r0 + S, :],
                                            in0=eup[r0:r0 + S, 0:D],
                                            scalar1=den_t[r0:r0 + S, :],
                                            scalar2=None, op0=ALU.mult)
            euT = sp.tile([128, KD, 128], BF16, tag="euT")
            pe_t(euT, eu_sb, id_b)

            # ---- batched GRU (rows 32*bl + s) ----
            gx = psg.tile([128, 3 * D], F32, tag="gates")
            for blk, w in ((0, 512), (512, 256)):
                for kd in range(KD):
                    nc.tensor.matmul(gx[:, blk:blk + w], lhsT=euT[:, kd, :],
                                     rhs=wvih_b[:, kd, blk:blk + w],
                                     start=(kd == 0), stop=False)
                nc.tensor.matmul(gx[:, blk:blk + w], lhsT=ones_row,
                                 rhs=gxbias_row[:, blk:blk + w],
                                 start=False, stop=True)
            gxs = sp.tile([128, 3 * D], F32, tag="gxs")
            nc.vector.tensor_copy(out=gxs, in_=gx)
            slT = sp.tile([128, KD, 128], BF16, tag="slT")
            pe_t(slT, slots_r, id_f)
            gh = psg.tile([128, 3 * D], F32, tag="gates")
            for blk, w in ((0, 512), (512, 256)):
                for kd in range(KD):
                    nc.tensor.matmul(gh[:, blk:blk + w], lhsT=slT[:, kd, :],
                                     rhs=whh_b[:, kd, blk:blk + w],
                                     start=(kd == 0), stop=False)
                nc.tensor.matmul(gh[:, blk:blk + w], lhsT=ones_row,
                                 rhs=ghbias_row[:, blk:blk + w],
                                 start=False, stop=True)
            rz = sp.tile([128, 2 * D], F32, tag="rz")
            nc.vector.tensor_tensor(out=rz, in0=gxs[:, 0:2 * D],
                                    in1=gh[:, 0:2 * D], op=ALU.add)
            nc.scalar.activation(out=rz, in_=rz, func=AF.Sigmoid)
            nsb = sp.tile([128, D], F32, tag="nsb")
            nc.vector.tensor_tensor(out=nsb, in0=rz[:, 0:D],
                                    in1=gh[:, 2 * D:3 * D], op=ALU.mult)
            nc.vector.tensor_tensor(out=nsb, in0=nsb, in1=gxs[:, 2 * D:3 * D],
                                    op=ALU.add)
            nc.scalar.activation(out=nsb, in_=nsb, func=AF.Tanh)
            dlt = sp.tile([128, D], F32, tag="dlt")
            nc.vector.tensor_tensor(out=dlt, in0=slots_r, in1=nsb,
                                    op=ALU.subtract)
            nc.vector.tensor_tensor(out=dlt, in0=dlt, in1=rz[:, D:2 * D],
                                    op=ALU.mult)
            nc.vector.tensor_tensor(out=slots_r, in0=nsb, in1=dlt, op=ALU.add)

            # ---- batched MLP with pre-LN; h1 produced transposed ----
            ffb = sp.tile([128, D], BF16, tag="ffb")
            slot_ln_cast(slots_r, ffb)
            ffT = sp.tile([128, KD, 128], BF16, tag="ffT")
            pe_t(ffT, ffb, id_b)
            h1T = sp.tile([128, KH, 128], BF16, tag="h1T")
            for ht in range(KH):
                psh = ps.tile([128, 128], F32, tag="small")
                for kd in range(KD):
                    nc.tensor.matmul(psh, lhsT=w1p_b[:, kd, ts(ht, 128)],
                                     rhs=ffT[:, kd, :],
                                     start=(kd == 0), stop=(kd == KD - 1))
                if has_b1:
                    nc.scalar.activation(out=h1T[:, ht, :], in_=psh,
                                         func=AF.Relu,
                                         bias=b1p_col[:, ht, :], scale=1.0)
                else:
                    nc.scalar.activation(out=h1T[:, ht, :], in_=psh,
                                         func=AF.Relu)
            ps2 = ps.tile([128, D], F32, tag="small")
            for kh in range(KH):
                nc.tensor.matmul(ps2, lhsT=h1T[:, kh, :], rhs=w2_b[:, kh, :],
                                 start=(kh == 0),
                                 stop=(kh == KH - 1 and not has_b2))
            if has_b2:
                nc.tensor.matmul(ps2, lhsT=ones_row, rhs=b2_row,
                                 start=False, stop=True)
            nc.vector.tensor_tensor(out=slots_r, in0=slots_r, in1=ps2,
                                    op=ALU.add)

        for bl in range(ROUND):
            nc.sync.dma_start(out=out_dram[:, rnd * ROUND + bl, :],
                              in_=slots_r[32 * bl:32 * bl + S, :])


def host_fold(w):
    """Fold LN affine params + projection chains on the host (numpy, fp32)."""
    g_in, b_in = w["g_in"], w["b_in"]
    wkp = g_in[:, None] * w["wk"]
    wvp = g_in[:, None] * w["wv"]
    fw = {
        "wkpT": np.ascontiguousarray(wkp.T),
        "wqp": w["g_sl"][:, None] * w["wq"],
        "wvih": wvp @ w["w_ih"],
        "whh": w["w_hh"],
        "w1p": w["g_ff"][:, None] * w["mlp_w1"],
        "w2": w["mlp_w2"],
        "gxbias": (b_in @ w["wv"] + w["bv"]) @ w["w_ih"] + w["b_ih"],
        "ghbias": w["b_hh"],
    }
    fw = {k: np.ascontiguousarray(v).astype(BF) for k, v in fw.items()}
    bkp = (b_in @ w["wk"] + w["bk"]).astype(np.float32)
    bqp = (w["b_sl"] @ w["wq"] + w["bq"]).astype(np.float32)
    b1p = (w["b_ff"] @ w["mlp_w1"] + w["mlp_b1"]).astype(np.float32)
    b2 = w["mlp_b2"].astype(np.float32)
    flags = (bool(np.any(bkp)), bool(np.any(bqp)), bool(np.any(b1p)),
             bool(np.any(b2)))
    if flags[0]:
        fw["bkp"] = bkp
    if flags[1]:
        fw["bqp"] = bqp
    if flags[2]:
        fw["b1p"] = b1p
    if flags[3]:
        fw["b2"] = b2.astype(BF)
    return fw, flags


_NC_CACHE = {}


def get_nc(flags):
    if flags not in _NC_CACHE:
        _NC_CACHE[flags] = _build(flags)
    return _NC_CACHE[flags]


def prepare(inputs, slots, **w):
    """Returns (nc, in_maps) for the 8-core SPMD launch."""
    inputs = np.ascontiguousarray(np.asarray(inputs, np.float32))
    slots = np.ascontiguousarray(np.asarray(slots, np.float32))
    w = {k: np.asarray(v, np.float32) for k, v in w.items()}
    fw, flags = host_fold(w)
    nc = get_nc(flags)
    n_cores = 8
    bs = inputs.shape[0] // n_cores
    in_maps = []
    for c in range(n_cores):
        m = dict(fw)
        m["inputs"] = inputs[c * bs:(c + 1) * bs]
        m["slots"] = np.ascontiguousarray(slots[:, c * bs:(c + 1) * bs, :])
        in_maps.append(m)
    return nc, in_maps


def kernel(inputs, slots, **w):
    nc, in_maps = prepare(inputs, slots, **w)
    res = run_bass_kernel_spmd(nc, in_maps, core_ids=list(range(len(in_maps))))
    out = np.concatenate([r["out"] for r in res.results], axis=1)
    return np.ascontiguousarray(out.astype(np.float32))


if __name__ == "__main__":
    nc = _build((False, False, False, False))
    print("built ok; instructions:", len(nc.inst_map))
